# revision 8
# baseline (speedup 1.0000x reference)
"""nn_GCNConv Trainium2 Bass kernel (8 NeuronCores, SPMD, no collectives).

Computation: out = segment_sum(features[src], dst, N) @ W + b
  features [10000,128] f32, edge_index [2,640000] i64, W [128,256], b [256]

Strategy (dense-adjacency streaming; dst-node sharding -> no cross-core
reduce):
  - Host builds the edge-count matrix A[dst, src] (pure index
    preprocessing: A = the sum of per-edge one-hots) and slices 1280 dst
    rows per core. Device receives A^T [10240 src x 1280 dst] as fp8e4
    (counts <= ~4 are exactly representable), partition-major in DRAM so
    every DMA group is 128 large contiguous descriptors, split into dst
    phases [0:1024) and [1024:1280) so phase-0 projections overlap the
    phase-1 stream.
  - Device keeps all features SBUF-resident as bf16 chunk-stationary
    weights: agg^T[f, dst] = sum_k feat_chunk_k^T @ A^T_chunk_k on PE
    (mixed bf16 x fp8 matmul, f32 PSUM accumulation over 80 src chunks).
    Feature groups load on the ACT HWDGE ring while A^T streams on the
    SP ring; group sizes are staged (1,1,2,4,8...) so the first matmul
    starts as early as possible.
  - Projection out = agg @ W + b uses bf16 PE matmuls (agg requantized
    on the PSUM->SBUF copy) + DVE bias add per 128-dst window; host
    concatenates the per-core [1280,256] outputs, truncates to 10000 rows.
"""

import sys

import numpy as np

_TRN_REPO = "/opt/trn_rl_repo"
if _TRN_REPO not in sys.path:
    sys.path.insert(0, _TRN_REPO)

import ml_dtypes  # noqa: E402

import concourse.bass as bass  # noqa: E402
import concourse.mybir as mybir  # noqa: E402
import concourse.tile as tile  # noqa: E402
from concourse import bacc, bass_utils  # noqa: E402

# ---------------------------------------------------------------------------
# Workaround: this walrus build rejects >1 sync-wait on a CTRL instruction
# ("Too many sync wait commands"). Tile's tail drain attaches a wait for every
# live sem lane to one InstDrain; chunk them across single-wait nops instead.
import re as _re  # noqa: E402

import bass_rust as _bass_rust  # noqa: E402


def _clock_ticks(vc):
    m = _re.search(r"\[([0-9, ]*)\]", repr(vc))
    return [int(x) for x in m.group(1).split(",")] if m.group(1).strip() else []


def _drain_and_barrier(self, tick_clock, wait_clock):
    ticks = _clock_ticks(tick_clock.global_clock)
    nz = [(i, t) for i, t in enumerate(ticks) if t > 0]
    for i, t in nz:
        vc = _bass_rust.VectorClock()
        vc.require_at_least(i, t)
        nop = self.nc.sync.nop(nofuse=True, hint="tail_wait")
        wait_clock.add_sem_waits(nop.ins, tile.ScopedClock({None: vc}))
    self.nc.sync.drain()  # waits already carried by the nops (SP FIFO order)
    self.nc.all_engine_barrier()
    assert self.sems is not None
    popped = self.nc._tile_sem_poison_stack.pop()
    assert popped is self._sem_poison
    self.nc.clear_and_free_semaphores(list(self.sems.allocated().values()))
    self.nc.all_engine_barrier()


tile.TileContext._drain_and_barrier = _drain_and_barrier
# ---------------------------------------------------------------------------

P = 128
C_IN = 128
C_OUT = 256
N_NODES = 10000
N_CORES = 8
WPC = 10                 # dst windows (of 128 nodes) per core
DPC = WPC * P            # dst nodes per core = 1280
D0 = 1024                # phase-0 dst columns (8 windows)
D1 = DPC - D0            # phase-1 dst columns (2 windows)
KCH = 80                 # src chunks (10240 padded rows)
NPAD = KCH * P           # 10240
GROUPS = [1, 1, 2, 4] + [8] * 9   # staged chunk-group sizes, sum = 80


def _build_kernel():
    nc = bacc.Bacc("TRN2")
    dt = mybir.dt
    at_dt = dt.float8e4

    # All streamed tensors are partition-major in DRAM: [128, chunks*cols]
    # with chunk-in-group contiguous per partition row -> each group DMA is
    # 128 large contiguous descriptors.
    featc_d = nc.dram_tensor("featc", [P, KCH * C_IN], dt.bfloat16, kind="ExternalInput")
    at0_d = nc.dram_tensor("at0", [P, KCH * D0], at_dt, kind="ExternalInput")
    at1_d = nc.dram_tensor("at1", [P, KCH * D1], at_dt, kind="ExternalInput")
    w_d = nc.dram_tensor("w", [C_IN, C_OUT], dt.bfloat16, kind="ExternalInput")
    bb_d = nc.dram_tensor("bb", [P, C_OUT], dt.float32, kind="ExternalInput")
    out_d = nc.dram_tensor("out", [DPC, C_OUT], dt.float32, kind="ExternalOutput")

    with tile.TileContext(nc) as tc:
        with (
            tc.tile_pool(name="consts", bufs=1) as cpool,
            tc.tile_pool(name="at0", bufs=4) as at0pool,
            tc.tile_pool(name="at1", bufs=4) as at1pool,
            tc.tile_pool(name="agg", bufs=1) as apool,
            tc.tile_pool(name="outs", bufs=3) as opool,
            tc.tile_pool(name="acc", bufs=1, space="PSUM") as accp,
            tc.tile_pool(name="pso", bufs=2, space="PSUM") as psop,
        ):
            w_s = cpool.tile([P, C_OUT], dt.bfloat16, tag="w")
            bb_s = cpool.tile([P, C_OUT], dt.float32, tag="bb")
            nc.scalar.dma_start(out=w_s[:], in_=w_d[:])
            nc.scalar.dma_start(out=bb_s[:], in_=bb_d[:])

            # per-group feature tiles (persist; loaded once on the ACT ring)
            ftiles = []
            off = 0
            for gi, gsz in enumerate(GROUPS):
                ft = cpool.tile([P, gsz, C_IN], dt.bfloat16, tag=f"f{gi}")
                nc.scalar.dma_start(
                    out=ft[:].rearrange("p a c -> p (a c)"),
                    in_=featc_d[:, off * C_IN : (off + gsz) * C_IN],
                )
                ftiles.append((off, ft))
                off += gsz

            def feat_chunk(a):
                for off, ft in reversed(ftiles):
                    if a >= off:
                        return ft[:, a - off, :]
                raise AssertionError(a)

            accA = accp.tile([P, 512], dt.float32, tag="accA")
            accB = accp.tile([P, 512], dt.float32, tag="accB")
            accC = accp.tile([P, 256], dt.float32, tag="accC")

            aggt_s = apool.tile([P, DPC], dt.bfloat16, tag="aggt")

            def proj(w):
                out_p = psop.tile([P, C_OUT], dt.float32, tag="outp")
                nc.tensor.matmul(
                    out_p[:],
                    lhsT=aggt_s[:, w * P : (w + 1) * P],
                    rhs=w_s[:],
                    start=True,
                    stop=True,
                )
                out_t = opool.tile([P, C_OUT], dt.float32, tag="outt")
                nc.vector.tensor_add(out_t[:], out_p[:], bb_s[:])
                nc.sync.dma_start(out=out_d[w * P : (w + 1) * P, :], in_=out_t[:])

            # ---- phase 0: dst columns [0:1024) ----
            off = 0
            for gi, gsz in enumerate(GROUPS):
                at_t = at0pool.tile([P, 8, D0], at_dt, tag="at0")
                nc.sync.dma_start(
                    out=at_t[:, :gsz, :].rearrange("p a d -> p (a d)"),
                    in_=at0_d[:, off * D0 : (off + gsz) * D0],
                )
                for r in range(gsz):
                    a = off + r
                    start = a == 0
                    stop = a == KCH - 1
                    nc.tensor.matmul(
                        accA[:], lhsT=feat_chunk(a), rhs=at_t[:, r, 0:512],
                        start=start, stop=stop,
                    )
                    nc.tensor.matmul(
                        accB[:], lhsT=feat_chunk(a), rhs=at_t[:, r, 512:1024],
                        start=start, stop=stop,
                    )
                off += gsz

            nc.scalar.copy(aggt_s[:, 0:512], accA[:])
            nc.scalar.copy(aggt_s[:, 512:1024], accB[:])

            # ---- phase 1: dst columns [1024:1280), with phase-0
            # projections interleaved so they hide under the stream ----
            projected = 0
            off = 0
            for gi, gsz in enumerate(GROUPS):
                at_t = at1pool.tile([P, 8, D1], at_dt, tag="at1")
                nc.sync.dma_start(
                    out=at_t[:, :gsz, :].rearrange("p a d -> p (a d)"),
                    in_=at1_d[:, off * D1 : (off + gsz) * D1],
                )
                for r in range(gsz):
                    a = off + r
                    nc.tensor.matmul(
                        accC[:], lhsT=feat_chunk(a), rhs=at_t[:, r, :],
                        start=(a == 0), stop=(a == KCH - 1),
                    )
                    if a >= 8 and a % 8 == 0 and projected < 8:
                        proj(projected)
                        projected += 1
                off += gsz
            while projected < 8:
                proj(projected)
                projected += 1

            nc.scalar.copy(aggt_s[:, 1024:1280], accC[:])
            proj(8)
            proj(9)

    nc.compile()
    return nc


def _prep_inputs(features, edge_index, W, b, n_cores: int, wpc: int):
    """Host-side sharding: count-matrix build + per-core dst slices."""
    src = np.asarray(edge_index[0], dtype=np.int64)
    dst = np.asarray(edge_index[1], dtype=np.int64)

    ndpad = n_cores * wpc * P
    at_u8 = np.zeros((NPAD, ndpad), dtype=np.uint8)
    np.add.at(at_u8, (src, dst), 1)

    lut = (
        np.arange(256, dtype=np.float32)
        .astype(ml_dtypes.float8_e4m3)
        .view(np.uint8)
    )
    at_all = np.take(lut, at_u8).view(ml_dtypes.float8_e4m3)
    # partition-major: [128, KCH * cols]
    at_pm = np.ascontiguousarray(
        at_all.reshape(KCH, P, ndpad).transpose(1, 0, 2)
    )  # [128, KCH, ndpad]

    feat_np = np.zeros((NPAD, C_IN), dtype=np.float32)
    feat_np[:N_NODES] = np.asarray(features, dtype=np.float32)
    featc = np.ascontiguousarray(
        feat_np.reshape(KCH, P, C_IN)
        .transpose(1, 0, 2)
        .reshape(P, KCH * C_IN)
        .astype(ml_dtypes.bfloat16)
    )
    w_np = np.ascontiguousarray(np.asarray(W, dtype=np.float32).astype(ml_dtypes.bfloat16))
    bb_np = np.tile(np.asarray(b, dtype=np.float32)[None, :], (P, 1))

    in_maps = []
    for c in range(n_cores):
        base = c * DPC
        at0 = np.ascontiguousarray(
            at_pm[:, :, base : base + D0].reshape(P, KCH * D0)
        )
        at1 = np.ascontiguousarray(
            at_pm[:, :, base + D0 : base + DPC].reshape(P, KCH * D1)
        )
        in_maps.append(
            {"featc": featc, "at0": at0, "at1": at1, "w": w_np, "bb": bb_np}
        )
    return in_maps, 0


_KERNEL_CACHE: dict = {}


def _get_kernel(nch: int = 0):
    key = (N_NODES, WPC, D0, KCH)
    if key not in _KERNEL_CACHE:
        _KERNEL_CACHE[key] = _build_kernel()
    return _KERNEL_CACHE[key]


def kernel(features, edge_index, W, b):
    features = np.asarray(features, dtype=np.float32)
    edge_index = np.asarray(edge_index)
    W = np.asarray(W, dtype=np.float32)
    b = np.asarray(b, dtype=np.float32)
    assert features.shape == (N_NODES, C_IN), features.shape
    assert W.shape == (C_IN, C_OUT) and b.shape == (C_OUT,)

    in_maps, nch = _prep_inputs(features, edge_index, W, b, N_CORES, WPC)
    nc = _get_kernel(nch)
    res = bass_utils.run_bass_kernel_spmd(nc, in_maps, core_ids=list(range(N_CORES)))
    out = np.concatenate([res.results[c]["out"] for c in range(N_CORES)], axis=0)
    return np.ascontiguousarray(out[:N_NODES]).astype(np.float32)


# revision 9
# speedup vs baseline: 1.0622x; 1.0622x over previous
"""nn_GCNConv Trainium2 Bass kernel (8 NeuronCores, SPMD, no collectives).

Computation: out = segment_sum(features[src], dst, N) @ W + b
  features [10000,128] f32, edge_index [2,640000] i64, W [128,256], b [256]

Strategy (dense-adjacency streaming; dst-node sharding -> no cross-core
reduce):
  - Host builds the edge-count matrix A[dst, src] (pure index
    preprocessing: A = the sum of per-edge one-hots) and slices 1280 dst
    rows per core. Device receives A^T [10240 src x 1280 dst] as fp8e4
    (counts <= ~4 are exactly representable), partition-major in DRAM so
    every DMA group is 128 large contiguous descriptors, split into dst
    phases [0:1024) and [1024:1280) so phase-0 projections overlap the
    phase-1 stream.
  - Device keeps all features SBUF-resident as bf16 chunk-stationary
    weights: agg^T[f, dst] = sum_k feat_chunk_k^T @ A^T_chunk_k on PE
    (mixed bf16 x fp8 matmul, f32 PSUM accumulation over 80 src chunks).
    Feature groups load on the ACT HWDGE ring while A^T streams on the
    SP ring; group sizes are staged (1,1,2,4,8...) so the first matmul
    starts as early as possible.
  - Projection out = agg @ W + b uses bf16 PE matmuls (agg requantized
    on the PSUM->SBUF copy) + DVE bias add per 128-dst window; host
    concatenates the per-core [1280,256] outputs, truncates to 10000 rows.
"""

import sys

import numpy as np

_TRN_REPO = "/opt/trn_rl_repo"
if _TRN_REPO not in sys.path:
    sys.path.insert(0, _TRN_REPO)

import ml_dtypes  # noqa: E402

import concourse.bass as bass  # noqa: E402
import concourse.mybir as mybir  # noqa: E402
import concourse.tile as tile  # noqa: E402
from concourse import bacc, bass_utils  # noqa: E402

# ---------------------------------------------------------------------------
# Workaround: this walrus build rejects >1 sync-wait on a CTRL instruction
# ("Too many sync wait commands"). Tile's tail drain attaches a wait for every
# live sem lane to one InstDrain; chunk them across single-wait nops instead.
import re as _re  # noqa: E402

import bass_rust as _bass_rust  # noqa: E402


def _clock_ticks(vc):
    m = _re.search(r"\[([0-9, ]*)\]", repr(vc))
    return [int(x) for x in m.group(1).split(",")] if m.group(1).strip() else []


def _drain_and_barrier(self, tick_clock, wait_clock):
    ticks = _clock_ticks(tick_clock.global_clock)
    nz = [(i, t) for i, t in enumerate(ticks) if t > 0]
    for i, t in nz:
        vc = _bass_rust.VectorClock()
        vc.require_at_least(i, t)
        nop = self.nc.sync.nop(nofuse=True, hint="tail_wait")
        wait_clock.add_sem_waits(nop.ins, tile.ScopedClock({None: vc}))
    self.nc.sync.drain()  # waits already carried by the nops (SP FIFO order)
    self.nc.all_engine_barrier()
    assert self.sems is not None
    popped = self.nc._tile_sem_poison_stack.pop()
    assert popped is self._sem_poison
    self.nc.clear_and_free_semaphores(list(self.sems.allocated().values()))
    self.nc.all_engine_barrier()


tile.TileContext._drain_and_barrier = _drain_and_barrier
# ---------------------------------------------------------------------------

P = 128
C_IN = 128
C_OUT = 256
N_NODES = 10000
N_CORES = 8
WPC = 10                 # dst windows (of 128 nodes) per core
DPC = WPC * P            # dst nodes per core = 1280
D0 = 1024                # phase-0 dst columns (8 windows)
D1 = DPC - D0            # phase-1 dst columns (2 windows)
KCH = 79                 # src chunks (10112 padded rows)
NPAD = KCH * P           # 10112
GROUPS = [1, 1, 2, 4] + [8] * 8 + [7]   # staged chunk-group sizes, sum = 79
AT1_SPLIT = 4            # at1 resident-load sub-DMAs


def _build_kernel():
    nc = bacc.Bacc("TRN2")
    dt = mybir.dt
    at_dt = dt.float8e4

    # All streamed tensors are partition-major in DRAM: [128, chunks*cols]
    # with chunk-in-group contiguous per partition row -> each group DMA is
    # 128 large contiguous descriptors.
    featc_d = nc.dram_tensor("featc", [P, KCH * C_IN], dt.bfloat16, kind="ExternalInput")
    at0_d = nc.dram_tensor("at0", [P, KCH * D0], at_dt, kind="ExternalInput")
    at1_d = nc.dram_tensor("at1", [P, KCH * D1], at_dt, kind="ExternalInput")
    w_d = nc.dram_tensor("w", [C_IN, C_OUT], dt.bfloat16, kind="ExternalInput")
    bb_d = nc.dram_tensor("bb", [P, C_OUT], dt.float32, kind="ExternalInput")
    out_d = nc.dram_tensor("out", [DPC, C_OUT], dt.float32, kind="ExternalOutput")

    with tile.TileContext(nc) as tc:
        with (
            tc.tile_pool(name="consts", bufs=1) as cpool,
            tc.tile_pool(name="at0", bufs=4) as at0pool,
            tc.tile_pool(name="agg", bufs=1) as apool,
            tc.tile_pool(name="outs", bufs=3) as opool,
            tc.tile_pool(name="acc", bufs=1, space="PSUM") as accp,
            tc.tile_pool(name="pso", bufs=3, space="PSUM") as psop,
        ):
            w_s = cpool.tile([P, C_OUT], dt.bfloat16, tag="w")
            bb_s = cpool.tile([P, C_OUT], dt.float32, tag="bb")
            nc.scalar.dma_start(out=w_s[:], in_=w_d[:])
            nc.scalar.dma_start(out=bb_s[:], in_=bb_d[:])

            # per-group feature tiles (persist; loaded once on the ACT ring)
            ftiles = []
            off = 0
            for gi, gsz in enumerate(GROUPS):
                ft = cpool.tile([P, gsz, C_IN], dt.bfloat16, tag=f"f{gi}")
                nc.scalar.dma_start(
                    out=ft[:].rearrange("p a c -> p (a c)"),
                    in_=featc_d[:, off * C_IN : (off + gsz) * C_IN],
                )
                ftiles.append((off, ft))
                off += gsz

            def feat_chunk(a):
                for off, ft in reversed(ftiles):
                    if a >= off:
                        return ft[:, a - off, :]
                raise AssertionError(a)

            accA = accp.tile([P, 512], dt.float32, tag="accA")
            accB = accp.tile([P, 512], dt.float32, tag="accB")
            accC = accp.tile([P, 256], dt.float32, tag="accC")

            aggt_s = apool.tile([P, DPC], dt.bfloat16, tag="aggt")

            def proj(w):
                out_p = psop.tile([P, C_OUT], dt.float32, tag="outp")
                nc.tensor.matmul(
                    out_p[:],
                    lhsT=aggt_s[:, w * P : (w + 1) * P],
                    rhs=w_s[:],
                    start=True,
                    stop=True,
                )
                out_t = opool.tile([P, C_OUT], dt.float32, tag="outt")
                nc.vector.tensor_add(out_t[:], out_p[:], bb_s[:])
                # ACT ring: keeps stores out of the SP ring's FIFO
                nc.scalar.dma_start(out=out_d[w * P : (w + 1) * P, :], in_=out_t[:])

            # phase-1 A^T is SBUF-resident: issued on the SP ring after all
            # at0 groups, in sub-DMAs sized to land just-in-time.
            at1_s = cpool.tile([P, KCH, D1], at_dt, tag="at1")

            # ---- phase 0: dst columns [0:1024) ----
            off = 0
            for gi, gsz in enumerate(GROUPS):
                at_t = at0pool.tile([P, 8, D0], at_dt, tag="at0")
                nc.sync.dma_start(
                    out=at_t[:, :gsz, :].rearrange("p a d -> p (a d)"),
                    in_=at0_d[:, off * D0 : (off + gsz) * D0],
                )
                for r in range(gsz):
                    a = off + r
                    start = a == 0
                    stop = a == KCH - 1
                    nc.tensor.matmul(
                        accA[:], lhsT=feat_chunk(a), rhs=at_t[:, r, 0:512],
                        start=start, stop=stop,
                    )
                    nc.tensor.matmul(
                        accB[:], lhsT=feat_chunk(a), rhs=at_t[:, r, 512:1024],
                        start=start, stop=stop,
                    )
                off += gsz

            # at1 resident load: 4 sub-DMAs of ~20 chunks each
            sub = (KCH + AT1_SPLIT - 1) // AT1_SPLIT
            at1_bounds = []
            for s0 in range(0, KCH, sub):
                s1 = min(s0 + sub, KCH)
                nc.sync.dma_start(
                    out=at1_s[:, s0:s1, :].rearrange("p a d -> p (a d)"),
                    in_=at1_d[:, s0 * D1 : s1 * D1],
                )
                at1_bounds.append((s0, s1))

            nc.scalar.copy(aggt_s[:, 0:512], accA[:])
            nc.scalar.copy(aggt_s[:, 512:1024], accB[:])

            # ---- phase 1: dst columns [1024:1280), DMA-free (at1 resident),
            # phase-0 projections interleaved so they hide under the stream --
            projected = 0
            for a in range(KCH):
                nc.tensor.matmul(
                    accC[:], lhsT=feat_chunk(a), rhs=at1_s[:, a, :],
                    start=(a == 0), stop=(a == KCH - 1),
                )
                if a >= 8 and a % 8 == 0 and projected < 8:
                    proj(projected)
                    projected += 1
            while projected < 8:
                proj(projected)
                projected += 1

            nc.scalar.copy(aggt_s[:, 1024:1280], accC[:])
            proj(8)
            proj(9)

    nc.compile()
    return nc


def _prep_inputs(features, edge_index, W, b, n_cores: int, wpc: int):
    """Host-side sharding: count-matrix build + per-core dst slices."""
    src = np.asarray(edge_index[0], dtype=np.int64)
    dst = np.asarray(edge_index[1], dtype=np.int64)

    ndpad = n_cores * wpc * P
    at_u8 = np.zeros((NPAD, ndpad), dtype=np.uint8)
    np.add.at(at_u8, (src, dst), 1)

    lut = (
        np.arange(256, dtype=np.float32)
        .astype(ml_dtypes.float8_e4m3)
        .view(np.uint8)
    )
    at_all = np.take(lut, at_u8).view(ml_dtypes.float8_e4m3)
    # partition-major: [128, KCH * cols]
    at_pm = np.ascontiguousarray(
        at_all.reshape(KCH, P, ndpad).transpose(1, 0, 2)
    )  # [128, KCH, ndpad]

    feat_np = np.zeros((NPAD, C_IN), dtype=np.float32)
    feat_np[:N_NODES] = np.asarray(features, dtype=np.float32)
    featc = np.ascontiguousarray(
        feat_np.reshape(KCH, P, C_IN)
        .transpose(1, 0, 2)
        .reshape(P, KCH * C_IN)
        .astype(ml_dtypes.bfloat16)
    )
    w_np = np.ascontiguousarray(np.asarray(W, dtype=np.float32).astype(ml_dtypes.bfloat16))
    bb_np = np.tile(np.asarray(b, dtype=np.float32)[None, :], (P, 1))

    in_maps = []
    for c in range(n_cores):
        base = c * DPC
        at0 = np.ascontiguousarray(
            at_pm[:, :, base : base + D0].reshape(P, KCH * D0)
        )
        at1 = np.ascontiguousarray(
            at_pm[:, :, base + D0 : base + DPC].reshape(P, KCH * D1)
        )
        in_maps.append(
            {"featc": featc, "at0": at0, "at1": at1, "w": w_np, "bb": bb_np}
        )
    return in_maps, 0


_KERNEL_CACHE: dict = {}


def _get_kernel(nch: int = 0):
    key = (N_NODES, WPC, D0, KCH)
    if key not in _KERNEL_CACHE:
        _KERNEL_CACHE[key] = _build_kernel()
    return _KERNEL_CACHE[key]


def kernel(features, edge_index, W, b):
    features = np.asarray(features, dtype=np.float32)
    edge_index = np.asarray(edge_index)
    W = np.asarray(W, dtype=np.float32)
    b = np.asarray(b, dtype=np.float32)
    assert features.shape == (N_NODES, C_IN), features.shape
    assert W.shape == (C_IN, C_OUT) and b.shape == (C_OUT,)

    in_maps, nch = _prep_inputs(features, edge_index, W, b, N_CORES, WPC)
    nc = _get_kernel(nch)
    res = bass_utils.run_bass_kernel_spmd(nc, in_maps, core_ids=list(range(N_CORES)))
    out = np.concatenate([res.results[c]["out"] for c in range(N_CORES)], axis=0)
    return np.ascontiguousarray(out[:N_NODES]).astype(np.float32)
